# revision 61
# baseline (speedup 1.0000x reference)
"""Self-contained Trainium2 Bass kernel for causal multi-head self-attention.

Problem (hardcoded): B=2, S=2048, D=1024, H=16 heads of width W=64, fp32.
  q,k,v = x@W* + b*; scores = causal(q k^T / 8); out = softmax(scores) v @ Wo + bo

Sharding: tensor-parallel over heads — core c owns heads (2c, 2c+1), i.e. a
128-column slice of Wq/Wk/Wv and a 128-row slice of Wo. Every core reads the
full (pre-transposed) x, computes q/k/v for its heads, runs causal attention,
and projects through its Wo slice; the host sums the 8 partial outputs (+bo).

Layout: everything stays transposed on-chip. xT [D, B*S] feeds the QKV
matmuls (contraction over D on partitions); qT/kT [128, B*S] feed scores
directly; scores are computed transposed [keys, queries] so softmax's key-sum
is a matmul with a ones-column appended to V (no partition reductions).
Softmax skips the max subtraction (scores ~ N(0,1); exp cannot overflow).

Performance structure (HW-measured ~230us vs the ~257us v1 baseline):
 - PE continuity: the whole program is emitted as interleaved generators —
   attention score/PV matmuls are woven with QKV-projection and
   output-projection matmuls so TensorE never waits on softmax (ACT) and
   stays out of the low DVFS p-state (PE needs ~3us of back-to-back work to
   reach 2.4GHz; any stall drops it to 1.2GHz).
 - per-instruction overhead is the dominant real-HW cost (each exp measured
   ~0.7us incl. ~0.25us fixed): both heads share one 2-bank score tile and
   ONE exp instruction per 128-key block (80 exps instead of 320).
 - PV, output projection, and the output partials run in bf16 (attention
   weights, v, h, Wo); scores/QKV stay fp32r. Halves output DMA and DVE
   eviction traffic; measured end-to-end rel-err ~3e-3 << 2e-2.
 - the PV lhsT is [v_h | ones] so the PV matmul deposits Z already broadcast
   across partitions (matmul time depends only on the moving dim) — softmax
   normalization is one reciprocal + one multiply per head, no DRAM
   roundtrip and no partition broadcast.
 - mask multiplies and the merged per-512-row output DMA run on the
   otherwise idle Pool/GpSimd engine (SWDGE); DVE only does PSUM evictions
   (tail projections alternate DVE/ACT-Copy, same act table as Exp).
 - hardware gotchas baked in: XBAR dma_start_transpose miscompiles for these
   shapes (silent every-2nd-row corruption) — v transposes via the PE with a
   f32r identity; strided 3D Pool tensor ops and the custom-DVE fast
   reciprocal also produced wrong results on HW and are avoided.
"""

import collections
import os
import sys

sys.path.insert(0, "/opt/trn_rl_repo")

from contextlib import ExitStack

import numpy as np

import concourse.tile as tile
from concourse import bacc, mybir

B, S, D, H = 2, 2048, 1024, 16
W = D // H            # 64
N = B * S             # 4096 rows
N_CORES = 8
HPC = H // N_CORES    # 2 heads per core
CD = HPC * W          # 128 columns of q/k/v per core
QC = 512              # query-chunk (moving dim of scores / PV / proj matmuls)
KC = 128              # key-chunk (partition dim of transposed score tiles)
SCALE = 1.0 / np.sqrt(W)

F32 = mybir.dt.float32
F32R = mybir.dt.float32r
BF16 = mybir.dt.bfloat16


def _build_program(loop_n=1):
    """Emit the per-core Bass/Tile program (same NEFF on all 8 cores).

    loop_n > 1 wraps the whole computation in a hardware loop that repeats
    it loop_n times — used only to measure per-iteration device time through
    the high-overhead dispatch path (the production kernel uses loop_n=1).
    """
    nc = bacc.Bacc("TRN2", target_bir_lowering=False, debug=False,
                   num_devices=N_CORES)

    xT_d = nc.dram_tensor("xT", [D, N], F32R, kind="ExternalInput").ap()
    wqkv_d = nc.dram_tensor("wqkv", [D, 3, CD], F32R, kind="ExternalInput").ap()
    wo_d = nc.dram_tensor("wo", [CD, D], BF16, kind="ExternalInput").ap()
    bqkv_d = nc.dram_tensor("bqkv", [CD, 3], F32, kind="ExternalInput").ap()
    # one [128,128] lower-triangle block mask (mask[k, q] = q >= k) — every
    # diagonal 128-key block sees the same local triangle; duplicated per
    # head so one multiply masks both heads' diagonal slices
    masks_d = nc.dram_tensor("masks", [KC, HPC, KC], BF16,
                             kind="ExternalInput").ap()
    ident_d = nc.dram_tensor("ident", [KC, KC], F32R, kind="ExternalInput").ap()
    out_d = nc.dram_tensor("out", [N, D], BF16, kind="ExternalOutput").ap()

    n_dc = D // KC            # 8 contraction chunks

    with tile.TileContext(nc) as tc, ExitStack() as ctx:
        def _bufs(name, default):
            return int(os.environ.get("KBUFS_" + name, default))

        singles = ctx.enter_context(tc.tile_pool(name="singles", bufs=1))
        xpool = ctx.enter_context(tc.tile_pool(name="xpool", bufs=_bufs("x", 4)))
        vtmp_p = ctx.enter_context(tc.tile_pool(name="vtmp", bufs=_bufs("v", 2)))
        epool = ctx.enter_context(tc.tile_pool(name="epool", bufs=_bufs("e", 7)))
        rzpool = ctx.enter_context(tc.tile_pool(name="rzpool", bufs=_bufs("r", 2)))
        fpool = ctx.enter_context(tc.tile_pool(name="fpool", bufs=_bufs("f", 3)))
        ppool = ctx.enter_context(tc.tile_pool(
            name="ppool", bufs=_bufs("p", 2), space="PSUM"))
        spool = ctx.enter_context(tc.tile_pool(
            name="spool", bufs=_bufs("s", 2), space="PSUM"))
        opool = ctx.enter_context(tc.tile_pool(
            name="opool", bufs=_bufs("o", 2), space="PSUM"))

        # ---- resident tensors -------------------------------------------
        wqkv = singles.tile([KC, n_dc, 3, CD], F32R)
        for dc in range(n_dc):
            nc.sync.dma_start(
                out=wqkv[:, dc, :, :],
                in_=wqkv_d.rearrange("(dc p) i m -> p dc i m", p=KC)[:, dc],
            )
        wo_s = singles.tile([CD, D], BF16)
        nc.sync.dma_start(out=wo_s[:], in_=wo_d[:, :])
        bqkv_s = singles.tile([CD, 3], F32)
        nc.sync.dma_start(out=bqkv_s[:], in_=bqkv_d[:, :])
        masks_s = singles.tile([KC, HPC, KC], BF16)
        nc.sync.dma_start(out=masks_s[:], in_=masks_d[:, :, :])
        ident_s = singles.tile([KC, KC], F32R)
        nc.sync.dma_start(out=ident_s[:], in_=ident_d[:, :])
        qT = singles.tile([CD, N], F32R)       # q, transposed, both heads stacked
        kT = singles.tile([CD, N], F32R)
        hT = singles.tile([CD, N], BF16)       # normalized attention output
        # v in natural layout + a ones block per head: [key_part, batch,
        # key_chunk, head, 2W] with cols 0:W = v_h, cols W:2W = ones. The PV
        # lhsT for head h is [:, b, kc, h, :] = [v_h | ones] — one
        # contiguous free-dim run (matmul lhsT APs allow only one) — which
        # makes the PV matmul deposit Z (= sum of attention weights over
        # keys) into op rows W..2W-1, already broadcast across 64
        # partitions: softmax normalization needs no partition broadcast at
        # all (matmul time only depends on the moving dim, so the extra
        # output rows are free).
        vaug = singles.tile([KC, B, S // KC, HPC, 2 * W], BF16)
        for b in range(B):
            nc.vector.memset(vaug[:, b, :, :, W:2 * W], 1.0)

        # ---- phase Q: q/k/v projections for one 512-row chunk -----------
        # xt loads are dispatched ~2 windows before consumption (xpool holds
        # 4 tiles) so the transfer + semaphore latency never reaches PE
        xt_tiles = {}

        def dispatch_xt(rc):
            xt = xpool.tile([KC, n_dc, QC], F32R, tag="xt")
            nh = 4 if rc == 0 else 2   # finer first load so PE starts sooner
            per = n_dc // nh
            for half in range(nh):
                d0 = half * per
                nc.sync.dma_start(
                    out=xt[:, d0:d0 + per, :],
                    in_=xT_d.rearrange("(dc p) r -> p dc r", p=KC)[
                        :, d0:d0 + per, rc * QC:(rc + 1) * QC],
                )
            xt_tiles[rc] = xt

        # yields once per PE matmul so the scheduler can interleave
        def gen_qkv(rc):
            xt = xt_tiles.pop(rc)
            for i in range(3):
                pp = ppool.tile([KC, QC], F32, tag="mm")
                for dc in range(n_dc):
                    nc.tensor.matmul(
                        out=pp[:],
                        lhsT=wqkv[:, dc, i, :],
                        rhs=xt[:, dc, :],
                        start=(dc == 0),
                        stop=(dc == n_dc - 1),
                        skip_group_check=True,
                    )
                    yield
                # bias-add on DVE: ScalarE must stay parked on the Exp
                # table set — Identity lives in a different set and each
                # switch costs ~2.7us on hardware
                if i == 2:
                    vtmp = vtmp_p.tile([CD, QC], F32R)
                    nc.vector.tensor_scalar_add(
                        out=vtmp[:], in0=pp[:], scalar1=bqkv_s[:, 2:3])
                    # v into natural [key, head, w] layout: f32r PE
                    # transposes (1.5 cycles/row) into one PSUM tile, then a
                    # single merged DVE copy (downcast to bf16) into vaug's
                    # [v_h | ones] slots
                    b, kc0 = (rc * QC) // S, ((rc * QC) % S) // KC
                    tp = ppool.tile([KC, QC], F32R, tag="mm")
                    for t in range(QC // KC):
                        nc.tensor.transpose(
                            tp[:, t * KC:(t + 1) * KC],
                            vtmp[:, t * KC:(t + 1) * KC], ident_s[:])
                        yield
                    nc.vector.tensor_copy(
                        out=vaug[:, b, kc0:kc0 + 4, :, 0:W],
                        in_=tp[:].bitcast(F32).rearrange(
                            "p (t h w) -> p t h w", t=QC // KC, h=HPC))
                else:
                    dst = (qT if i == 0 else kT)[:, rc * QC:(rc + 1) * QC]
                    nc.vector.tensor_scalar_add(
                        out=dst, in0=pp[:], scalar1=bqkv_s[:, i:i + 1])

        # ---- phase A: attention for one 512-query chunk -----------------
        # PE stream per 128-key block: score matmul now, PV matmul ~2 blocks
        # later (so ACT's exp never stalls PE); 4*nkc+1 yields total
        def gen_attn(b, j):
            q0 = b * S + j * QC          # global row of this query chunk
            nkc = (j + 1) * (QC // KC)   # causal: key chunks 0 .. nkc-1
            ops = [opool.tile([2 * W, QC], F32, name="op")
                   for h in range(HPC)]
            pend = collections.deque()

            def flush_pv():
                kc, c0, et = pend.popleft()
                for h in range(HPC):
                    nc.tensor.matmul(
                        out=ops[h][:, c0:QC],
                        lhsT=vaug[:, b, kc, h, :],
                        rhs=et[:, h, c0:QC],
                        start=(kc == 0), stop=(kc == nkc - 1),
                        skip_group_check=True,
                    )
                    if kc == nkc - 1:
                        # normalize this head as soon as its accumulation
                        # closes (op rows W..2W-1 hold Z, already broadcast)
                        rz = rzpool.tile([W, QC], F32, name="rz")
                        nc.vector.reciprocal(rz[:], ops[h][W:2 * W, :])
                        nc.vector.tensor_mul(
                            hT[h * W:(h + 1) * W, q0:q0 + QC],
                            ops[h][0:W, :], rz[:])
                    yield

            for kc in range(nkc):
                dg = kc - (nkc - 4)  # >=0 on the 4 diagonal tiles
                c0 = KC * dg if dg > 0 else 0
                # queries < c0 precede every key of this block, so only
                # columns [c0:] are computed / accumulated. Both heads share
                # one 2-bank score tile and ONE exp instruction — ACT
                # per-instruction overhead and cross-engine semaphore hops
                # are the real bottleneck of the attention phase.
                sp = spool.tile([KC, HPC, QC], F32)
                for h in range(HPC):
                    nc.tensor.matmul(
                        out=sp[:, h, c0:QC],
                        lhsT=kT[h * W:(h + 1) * W,
                                b * S + kc * KC:b * S + (kc + 1) * KC],
                        rhs=qT[h * W:(h + 1) * W, q0 + c0:q0 + QC],
                        start=True, stop=True,
                        skip_group_check=True,
                    )
                    yield
                et = epool.tile([KC, HPC, QC], BF16)
                nc.scalar.activation(
                    out=et[:, :, c0:QC], in_=sp[:, :, c0:QC],
                    func=mybir.ActivationFunctionType.Exp,
                    scale=float(SCALE),
                )
                if dg >= 0:
                    # triangle-mask the 128-col block containing the
                    # diagonal (idle Pool engine; et is SBUF). One 2D mul
                    # per head — strided 3D Pool ops miscompile on HW.
                    for h in range(HPC):
                        nc.gpsimd.tensor_mul(
                            et[:, h, c0:c0 + KC], et[:, h, c0:c0 + KC],
                            masks_s[:, 0, :])
                pend.append((kc, c0, et))
                if len(pend) > 2:
                    yield from flush_pv()
            while pend:
                yield from flush_pv()

        # ---- phase P: output projection for one 512-row chunk -----------
        # 8 yields; partial out staged in bf16, one merged DMA per chunk on
        # the Pool engine's SWDGE queue (keeps the SP queue for x loads).
        # act_evict=True alternates the PSUM evictions between DVE and ACT
        # (Copy is in the exp table set, so no table switch) — used for the
        # tail projections that run after the last exp, where DVE's eviction
        # rate would otherwise bound the drain.
        def gen_proj(b, j, act_evict=False):
            q0 = b * S + j * QC
            ft = fpool.tile([KC, QC // KC, D], BF16)
            for t in range(QC // KC):
                r0 = q0 + t * KC
                for cc in range(D // QC):
                    pp = ppool.tile([KC, QC], F32, tag="mm")
                    nc.tensor.matmul(
                        out=pp[:],
                        lhsT=hT[:, r0:r0 + KC],
                        rhs=wo_s[:, cc * QC:(cc + 1) * QC],
                        start=True, stop=True,
                        skip_group_check=True,
                    )
                    dst = ft[:, t, cc * QC:(cc + 1) * QC]
                    if act_evict and (t * 2 + cc) % 2:
                        nc.scalar.activation(
                            out=dst, in_=pp[:],
                            func=mybir.ActivationFunctionType.Copy)
                    else:
                        nc.vector.tensor_copy(out=dst, in_=pp[:])
                    yield
            out_eng = nc.sync if os.environ.get("KOUT_SYNC") else nc.gpsimd
            out_eng.dma_start(
                out=out_d[q0:q0 + QC, :].rearrange("(t p) c -> p t c", p=KC),
                in_=ft[:])

        # ---- scheduler: weave attention with QKV/proj filler matmuls ----
        # fillers: list of (gen, n_yields, speed). speed=2 front-loads the
        # filler into the window's first half (QKV results are needed by the
        # NEXT window's first scores, so they must not land at window end)
        def weave(main_gen, mlen, fillers):
            state = [[g, n, sp, 0] for g, n, sp in fillers]
            done_m = 0
            for _ in main_gen:
                done_m += 1
                for st in state:
                    g, n, sp, done = st
                    want = min(n, int(n * done_m * sp) // mlen)
                    while st[3] < want and next(g, None) is not None:
                        st[3] += 1
            for g, _, _, _ in state:
                for _ in g:
                    pass

        def emit_all():
            for rc in range(3):
                dispatch_xt(rc)
            for _ in gen_qkv(0):
                pass
            sched = [
                ((0, 0), 3, lambda: [(gen_qkv(1), 28, 2)]),
                ((0, 1), 4, lambda: [(gen_qkv(2), 28, 2), (gen_proj(0, 0), 8, 1)]),
                ((0, 2), 5, lambda: [(gen_qkv(3), 28, 2), (gen_proj(0, 1), 8, 1)]),
                ((0, 3), 6, lambda: [(gen_qkv(4), 28, 2), (gen_proj(0, 2), 8, 1)]),
                ((1, 0), 7, lambda: [(gen_qkv(5), 28, 2)]),
                ((1, 1), None, lambda: [(gen_qkv(6), 28, 2), (gen_proj(0, 3), 8, 1)]),
                ((1, 2), None, lambda: [(gen_qkv(7), 28, 2), (gen_proj(1, 0), 8, 1)]),
                ((1, 3), None, lambda: [(gen_proj(1, 1), 8, 1),
                                        (gen_proj(1, 2), 8, 0.5)]),
            ]
            for (b, j), pre, fills in sched:
                if pre is not None:
                    dispatch_xt(pre)
                nkc = (j + 1) * (QC // KC)
                weave(gen_attn(b, j), 4 * nkc, fills())
            for _ in gen_proj(1, 3, act_evict=True):
                pass

        if loop_n == 1:
            emit_all()
        else:
            with tc.For_i(0, loop_n, 1):
                emit_all()

    nc.compile()
    return nc


_CACHE = {}


def _get_program(loop_n=1):
    key = ("nc", loop_n)
    if key not in _CACHE:
        _CACHE[key] = _build_program(loop_n)
    return _CACHE[key]


def _make_masks():
    k = np.arange(KC, dtype=np.int32)[:, None]
    q = np.arange(KC, dtype=np.int32)[None, :]
    return (q >= k).astype(np.float32)


def make_in_maps(x, Wq, bq, Wk, bk, Wv, bv, Wo):
    bf = mybir.dt.np(BF16)
    x = np.asarray(x, np.float32).reshape(N, D)
    xT = np.ascontiguousarray(x.T)
    masks = np.repeat(_make_masks()[:, None, :], HPC, axis=1).astype(bf)
    Wq, Wk, Wv = (np.asarray(a, np.float32) for a in (Wq, Wk, Wv))
    Wo = np.asarray(Wo, np.float32)
    bq, bk, bv = (np.asarray(a, np.float32) for a in (bq, bk, bv))
    in_maps = []
    for c in range(N_CORES):
        sl = slice(c * CD, (c + 1) * CD)
        in_maps.append({
            "xT": xT,
            "wqkv": np.ascontiguousarray(
                np.stack([Wq[:, sl], Wk[:, sl], Wv[:, sl]], axis=1)),
            "wo": np.ascontiguousarray(Wo[sl, :]).astype(bf),
            "bqkv": np.ascontiguousarray(
                np.stack([bq[sl], bk[sl], bv[sl]], axis=1)),
            "masks": masks,
            "ident": np.eye(KC, dtype=np.float32),
        })
    return in_maps


def _get_runner(loop_n=1):
    """Build (once) a cached jitted SPMD executable over the 8 cores.

    Mirrors concourse.bass2jax.run_bass_via_pjrt's multi-core branch, but
    caches the jitted callable so repeated calls skip re-tracing/compiling,
    and exposes input staging separately so executions can be timed with
    device-resident inputs.
    """
    rkey = ("runner", loop_n)
    if rkey in _CACHE:
        return _CACHE[rkey]
    import jax
    import jax.numpy as jnp
    from jax.sharding import Mesh, PartitionSpec, NamedSharding
    from jax.experimental.shard_map import shard_map
    from concourse import bass2jax
    from concourse import mybir as _mybir

    nc = _get_program(loop_n)
    bass2jax.install_neuronx_cc_hook()

    in_names, out_names, out_avals = [], [], []
    assert nc.dbg_addr is None
    part_name = (nc.partition_id_tensor.name
                 if nc.partition_id_tensor is not None else None)
    for alloc in nc.m.functions[0].allocations:
        if not isinstance(alloc, _mybir.MemoryLocationSet):
            continue
        name = alloc.memorylocations[0].name
        if alloc.kind == "ExternalInput":
            if name != part_name:
                in_names.append(name)
        elif alloc.kind == "ExternalOutput":
            out_names.append(name)
            out_avals.append(jax.core.ShapedArray(
                tuple(alloc.tensor_shape), _mybir.dt.np(alloc.dtype)))
    n_params = len(in_names)
    all_names = in_names + out_names
    if part_name is not None:
        all_names = all_names + [part_name]

    def _body(*args):
        operands = list(args)
        if part_name is not None:
            operands.append(bass2jax.partition_id_tensor())
        return tuple(bass2jax._bass_exec_p.bind(
            *operands,
            out_avals=tuple(out_avals),
            in_names=tuple(all_names),
            out_names=tuple(out_names),
            lowering_input_output_aliases=(),
            sim_require_finite=True,
            sim_require_nnan=True,
            nc=nc,
        ))

    devices = jax.devices()[:N_CORES]
    mesh = Mesh(np.asarray(devices), ("core",))
    nshard = NamedSharding(mesh, PartitionSpec("core"))
    n_outs = len(out_names)
    donate = tuple(range(n_params, n_params + n_outs))
    sharded = jax.jit(
        shard_map(_body, mesh=mesh,
                  in_specs=(PartitionSpec("core"),) * (n_params + n_outs),
                  out_specs=(PartitionSpec("core"),) * n_outs,
                  check_rep=False),
        donate_argnums=donate, keep_unused=True)

    zero_shapes = [(N_CORES * a.shape[0], *a.shape[1:]) for a in out_avals]
    zero_dtypes = [a.dtype for a in out_avals]
    make_zeros = jax.jit(
        lambda: tuple(jnp.zeros(s, d) for s, d in zip(zero_shapes, zero_dtypes)),
        out_shardings=(nshard,) * n_outs)

    def stage(in_maps):
        assert len(in_maps) == N_CORES
        concat = [np.concatenate([np.asarray(m[name]) for m in in_maps], axis=0)
                  for name in in_names]
        return [jax.device_put(a, nshard) for a in concat]

    def execute(staged):
        zeros = make_zeros()
        import jax as _jax
        _jax.block_until_ready(zeros)
        import time as _time
        t0 = _time.perf_counter()
        outs = sharded(*staged, *zeros)
        outs = _jax.block_until_ready(outs)
        dt = _time.perf_counter() - t0
        per_core = [
            {name: np.asarray(outs[i]).reshape(N_CORES, *out_avals[i].shape)[c]
             for i, name in enumerate(out_names)}
            for c in range(N_CORES)]
        return per_core, dt

    _CACHE[rkey] = (stage, execute)
    return _CACHE[rkey]


def run_cores(in_maps):
    """Execute the SPMD program; returns list of per-core {'out': partial}."""
    stage, execute = _get_runner()
    results, _ = execute(stage(in_maps))
    return results


def timed_runs(in_maps, n=8, loop_n=1):
    """Stage inputs once, execute n times, return list of wall durations (s)."""
    stage, execute = _get_runner(loop_n)
    staged = stage(in_maps)
    times = []
    for _ in range(n):
        _, dt = execute(staged)
        times.append(dt)
    return times


def kernel(x, seg, Wq, bq, Wk, bk, Wv, bv, Wo, bo):
    del seg  # unused by the reference computation
    in_maps = make_in_maps(x, Wq, bq, Wk, bk, Wv, bv, Wo)
    results = run_cores(in_maps)
    acc = np.zeros((N, D), np.float32)
    for r in results:
        acc += r["out"].astype(np.float32)
    out = acc + np.asarray(bo, np.float32)
    return out.reshape(B, S, D)


# revision 62
# speedup vs baseline: 1.0809x; 1.0809x over previous
"""Self-contained Trainium2 Bass kernel for causal multi-head self-attention.

Problem (hardcoded): B=2, S=2048, D=1024, H=16 heads of width W=64, fp32.
  q,k,v = x@W* + b*; scores = causal(q k^T / 8); out = softmax(scores) v @ Wo + bo

Sharding: tensor-parallel over heads — core c owns heads (2c, 2c+1), i.e. a
128-column slice of Wq/Wk/Wv and a 128-row slice of Wo. Every core reads the
full (pre-transposed) x, computes q/k/v for its heads, runs causal attention,
and projects through its Wo slice; the host sums the 8 partial outputs (+bo).

Layout: everything stays transposed on-chip. xT [D, B*S] feeds the QKV
matmuls (contraction over D on partitions); qT/kT [128, B*S] feed scores
directly; scores are computed transposed [keys, queries] so softmax's key-sum
is a matmul with a ones-column appended to V (no partition reductions).
Softmax skips the max subtraction (scores ~ N(0,1); exp cannot overflow).

Performance structure (HW-measured ~230us vs the ~257us v1 baseline):
 - PE continuity: the whole program is emitted as interleaved generators —
   attention score/PV matmuls are woven with QKV-projection and
   output-projection matmuls so TensorE never waits on softmax (ACT) and
   stays out of the low DVFS p-state (PE needs ~3us of back-to-back work to
   reach 2.4GHz; any stall drops it to 1.2GHz).
 - per-instruction overhead is the dominant real-HW cost (each exp measured
   ~0.7us incl. ~0.25us fixed): both heads share one 2-bank score tile and
   ONE exp instruction per 128-key block (80 exps instead of 320).
 - PV, output projection, and the output partials run in bf16 (attention
   weights, v, h, Wo); scores/QKV stay fp32r. Halves output DMA and DVE
   eviction traffic; measured end-to-end rel-err ~3e-3 << 2e-2.
 - the PV lhsT is [v_h | ones] so the PV matmul deposits Z already broadcast
   across partitions (matmul time depends only on the moving dim) — softmax
   normalization is one reciprocal + one multiply per head, no DRAM
   roundtrip and no partition broadcast.
 - mask multiplies and the merged per-512-row output DMA run on the
   otherwise idle Pool/GpSimd engine (SWDGE); DVE only does PSUM evictions
   (tail projections alternate DVE/ACT-Copy, same act table as Exp).
 - hardware gotchas baked in: XBAR dma_start_transpose miscompiles for these
   shapes (silent every-2nd-row corruption) — v transposes via the PE with a
   f32r identity; strided 3D Pool tensor ops and the custom-DVE fast
   reciprocal also produced wrong results on HW and are avoided.
"""

import collections
import os
import sys

sys.path.insert(0, "/opt/trn_rl_repo")

from contextlib import ExitStack

import numpy as np

import concourse.tile as tile
from concourse import bacc, mybir

B, S, D, H = 2, 2048, 1024, 16
W = D // H            # 64
N = B * S             # 4096 rows
N_CORES = 8
HPC = H // N_CORES    # 2 heads per core
CD = HPC * W          # 128 columns of q/k/v per core
QC = 512              # query-chunk (moving dim of scores / PV / proj matmuls)
KC = 128              # key-chunk (partition dim of transposed score tiles)
SCALE = 1.0 / np.sqrt(W)

F32 = mybir.dt.float32
F32R = mybir.dt.float32r
BF16 = mybir.dt.bfloat16


def _build_program(loop_n=1):
    """Emit the per-core Bass/Tile program (same NEFF on all 8 cores).

    loop_n > 1 wraps the whole computation in a hardware loop that repeats
    it loop_n times — used only to measure per-iteration device time through
    the high-overhead dispatch path (the production kernel uses loop_n=1).
    """
    nc = bacc.Bacc("TRN2", target_bir_lowering=False, debug=False,
                   num_devices=N_CORES)

    xT_d = nc.dram_tensor("xT", [D, N], F32R, kind="ExternalInput").ap()
    wqkv_d = nc.dram_tensor("wqkv", [D, 3, CD], F32R, kind="ExternalInput").ap()
    wo_d = nc.dram_tensor("wo", [CD, D], BF16, kind="ExternalInput").ap()
    bqkv_d = nc.dram_tensor("bqkv", [CD, 3], F32, kind="ExternalInput").ap()
    # one [128,128] lower-triangle block mask (mask[k, q] = q >= k) — every
    # diagonal 128-key block sees the same local triangle; duplicated per
    # head so one multiply masks both heads' diagonal slices
    masks_d = nc.dram_tensor("masks", [KC, HPC, KC], BF16,
                             kind="ExternalInput").ap()
    ident_d = nc.dram_tensor("ident", [KC, KC], F32R, kind="ExternalInput").ap()
    out_d = nc.dram_tensor("out", [N, D], BF16, kind="ExternalOutput").ap()

    n_dc = D // KC            # 8 contraction chunks

    with tile.TileContext(nc) as tc, ExitStack() as ctx:
        def _bufs(name, default):
            return int(os.environ.get("KBUFS_" + name, default))

        singles = ctx.enter_context(tc.tile_pool(name="singles", bufs=1))
        xpool = ctx.enter_context(tc.tile_pool(name="xpool", bufs=_bufs("x", 4)))
        vtmp_p = ctx.enter_context(tc.tile_pool(name="vtmp", bufs=_bufs("v", 2)))
        epool = ctx.enter_context(tc.tile_pool(name="epool", bufs=_bufs("e", 7)))
        rzpool = ctx.enter_context(tc.tile_pool(name="rzpool", bufs=_bufs("r", 2)))
        fpool = ctx.enter_context(tc.tile_pool(name="fpool", bufs=_bufs("f", 3)))
        ppool = ctx.enter_context(tc.tile_pool(
            name="ppool", bufs=_bufs("p", 2), space="PSUM"))
        spool = ctx.enter_context(tc.tile_pool(
            name="spool", bufs=_bufs("s", 2), space="PSUM"))
        opool = ctx.enter_context(tc.tile_pool(
            name="opool", bufs=_bufs("o", 2), space="PSUM"))

        # ---- resident tensors -------------------------------------------
        wqkv = singles.tile([KC, n_dc, 3, CD], F32R)
        for dc in range(n_dc):
            nc.sync.dma_start(
                out=wqkv[:, dc, :, :],
                in_=wqkv_d.rearrange("(dc p) i m -> p dc i m", p=KC)[:, dc],
            )
        wo_s = singles.tile([CD, D], BF16)
        nc.sync.dma_start(out=wo_s[:], in_=wo_d[:, :])
        bqkv_s = singles.tile([CD, 3], F32)
        nc.sync.dma_start(out=bqkv_s[:], in_=bqkv_d[:, :])
        masks_s = singles.tile([KC, HPC, KC], BF16)
        nc.sync.dma_start(out=masks_s[:], in_=masks_d[:, :, :])
        ident_s = singles.tile([KC, KC], F32R)
        nc.sync.dma_start(out=ident_s[:], in_=ident_d[:, :])
        qT = singles.tile([CD, N], F32R)       # q, transposed, both heads stacked
        kT = singles.tile([CD, N], F32R)
        hT = singles.tile([CD, N], BF16)       # normalized attention output
        # v in natural layout + a ones block per head: [key_part, batch,
        # key_chunk, head, 2W] with cols 0:W = v_h, cols W:2W = ones. The PV
        # lhsT for head h is [:, b, kc, h, :] = [v_h | ones] — one
        # contiguous free-dim run (matmul lhsT APs allow only one) — which
        # makes the PV matmul deposit Z (= sum of attention weights over
        # keys) into op rows W..2W-1, already broadcast across 64
        # partitions: softmax normalization needs no partition broadcast at
        # all (matmul time only depends on the moving dim, so the extra
        # output rows are free).
        vaug = singles.tile([KC, B, S // KC, HPC, 2 * W], BF16)
        for b in range(B):
            nc.vector.memset(vaug[:, b, :, :, W:2 * W], 1.0)

        # ---- phase Q: q/k/v projections for one 512-row chunk -----------
        # xt loads are dispatched ~2 windows before consumption (xpool holds
        # 4 tiles) so the transfer + semaphore latency never reaches PE
        xt_tiles = {}

        def dispatch_xt(rc):
            xt = xpool.tile([KC, n_dc, QC], F32R, tag="xt")
            nh = 4 if rc == 0 else 2   # finer first load so PE starts sooner
            per = n_dc // nh
            for half in range(nh):
                d0 = half * per
                nc.sync.dma_start(
                    out=xt[:, d0:d0 + per, :],
                    in_=xT_d.rearrange("(dc p) r -> p dc r", p=KC)[
                        :, d0:d0 + per, rc * QC:(rc + 1) * QC],
                )
            xt_tiles[rc] = xt

        # yields once per PE matmul so the scheduler can interleave
        def gen_qkv(rc):
            xt = xt_tiles.pop(rc)
            for i in range(3):
                pp = ppool.tile([KC, QC], F32, tag="mm")
                for dc in range(n_dc):
                    nc.tensor.matmul(
                        out=pp[:],
                        lhsT=wqkv[:, dc, i, :],
                        rhs=xt[:, dc, :],
                        start=(dc == 0),
                        stop=(dc == n_dc - 1),
                        skip_group_check=True,
                    )
                    yield
                # bias-add on DVE: ScalarE must stay parked on the Exp
                # table set — Identity lives in a different set and each
                # switch costs ~2.7us on hardware
                if i == 2:
                    vtmp = vtmp_p.tile([CD, QC], F32R)
                    nc.vector.tensor_scalar_add(
                        out=vtmp[:], in0=pp[:], scalar1=bqkv_s[:, 2:3])
                    # v into natural [key, head, w] layout: f32r PE
                    # transposes (1.5 cycles/row) into one PSUM tile, then a
                    # single merged DVE copy (downcast to bf16) into vaug's
                    # [v_h | ones] slots
                    b, kc0 = (rc * QC) // S, ((rc * QC) % S) // KC
                    tp = ppool.tile([KC, QC], F32R, tag="mm")
                    for t in range(QC // KC):
                        nc.tensor.transpose(
                            tp[:, t * KC:(t + 1) * KC],
                            vtmp[:, t * KC:(t + 1) * KC], ident_s[:])
                        yield
                    nc.vector.tensor_copy(
                        out=vaug[:, b, kc0:kc0 + 4, :, 0:W],
                        in_=tp[:].bitcast(F32).rearrange(
                            "p (t h w) -> p t h w", t=QC // KC, h=HPC))
                else:
                    dst = (qT if i == 0 else kT)[:, rc * QC:(rc + 1) * QC]
                    nc.vector.tensor_scalar_add(
                        out=dst, in0=pp[:], scalar1=bqkv_s[:, i:i + 1])

        # ---- phase A: attention for one 512-query chunk -----------------
        # PE stream per 128-key block: score matmul now, PV matmul ~2 blocks
        # later (so ACT's exp never stalls PE); 4*nkc+1 yields total
        def gen_attn(b, j):
            q0 = b * S + j * QC          # global row of this query chunk
            nkc = (j + 1) * (QC // KC)   # causal: key chunks 0 .. nkc-1
            ops = [opool.tile([2 * W, QC], F32, name="op")
                   for h in range(HPC)]
            pend = collections.deque()

            def flush_pv():
                kc, c0, et = pend.popleft()
                for h in range(HPC):
                    nc.tensor.matmul(
                        out=ops[h][:, c0:QC],
                        lhsT=vaug[:, b, kc, h, :],
                        rhs=et[:, h, c0:QC],
                        start=(kc == 0), stop=(kc == nkc - 1),
                        skip_group_check=True,
                    )
                    if kc == nkc - 1:
                        # normalize this head as soon as its accumulation
                        # closes (op rows W..2W-1 hold Z, already broadcast)
                        rz = rzpool.tile([W, QC], F32, name="rz")
                        nc.vector.reciprocal(rz[:], ops[h][W:2 * W, :])
                        nc.vector.tensor_mul(
                            hT[h * W:(h + 1) * W, q0:q0 + QC],
                            ops[h][0:W, :], rz[:])
                    yield

            for kc in range(nkc):
                dg = kc - (nkc - 4)  # >=0 on the 4 diagonal tiles
                c0 = KC * dg if dg > 0 else 0
                # queries < c0 precede every key of this block, so only
                # columns [c0:] are computed / accumulated. Both heads share
                # one 2-bank score tile and ONE exp instruction — ACT
                # per-instruction overhead and cross-engine semaphore hops
                # are the real bottleneck of the attention phase.
                sp = spool.tile([KC, HPC, QC], F32)
                for h in range(HPC):
                    nc.tensor.matmul(
                        out=sp[:, h, c0:QC],
                        lhsT=kT[h * W:(h + 1) * W,
                                b * S + kc * KC:b * S + (kc + 1) * KC],
                        rhs=qT[h * W:(h + 1) * W, q0 + c0:q0 + QC],
                        start=True, stop=True,
                        skip_group_check=True,
                    )
                    yield
                et = epool.tile([KC, HPC, QC], BF16)
                nc.scalar.activation(
                    out=et[:, :, c0:QC], in_=sp[:, :, c0:QC],
                    func=mybir.ActivationFunctionType.Exp,
                    scale=float(SCALE),
                )
                if dg >= 0:
                    # triangle-mask the 128-col block containing the
                    # diagonal (idle Pool engine; et is SBUF). One 2D mul
                    # per head — strided 3D Pool ops miscompile on HW.
                    for h in range(HPC):
                        nc.gpsimd.tensor_mul(
                            et[:, h, c0:c0 + KC], et[:, h, c0:c0 + KC],
                            masks_s[:, 0, :])
                pend.append((kc, c0, et))
                if len(pend) > 1:
                    yield from flush_pv()
            while pend:
                yield from flush_pv()

        # ---- phase P: output projection for one 512-row chunk -----------
        # 8 yields; partial out staged in bf16, one merged DMA per chunk on
        # the Pool engine's SWDGE queue (keeps the SP queue for x loads).
        # act_evict=True alternates the PSUM evictions between DVE and ACT
        # (Copy is in the exp table set, so no table switch) — used for the
        # tail projections that run after the last exp, where DVE's eviction
        # rate would otherwise bound the drain.
        def gen_proj(b, j, act_evict=False):
            q0 = b * S + j * QC
            ft = fpool.tile([KC, QC // KC, D], BF16)
            for t in range(QC // KC):
                r0 = q0 + t * KC
                for cc in range(D // QC):
                    pp = ppool.tile([KC, QC], F32, tag="mm")
                    nc.tensor.matmul(
                        out=pp[:],
                        lhsT=hT[:, r0:r0 + KC],
                        rhs=wo_s[:, cc * QC:(cc + 1) * QC],
                        start=True, stop=True,
                        skip_group_check=True,
                    )
                    dst = ft[:, t, cc * QC:(cc + 1) * QC]
                    if act_evict and (t * 2 + cc) % 2:
                        nc.scalar.activation(
                            out=dst, in_=pp[:],
                            func=mybir.ActivationFunctionType.Copy)
                    else:
                        nc.vector.tensor_copy(out=dst, in_=pp[:])
                    yield
            out_eng = nc.sync if os.environ.get("KOUT_SYNC") else nc.gpsimd
            out_eng.dma_start(
                out=out_d[q0:q0 + QC, :].rearrange("(t p) c -> p t c", p=KC),
                in_=ft[:])

        # ---- scheduler: weave attention with QKV/proj filler matmuls ----
        # fillers: list of (gen, n_yields, speed). speed=2 front-loads the
        # filler into the window's first half (QKV results are needed by the
        # NEXT window's first scores, so they must not land at window end)
        def weave(main_gen, mlen, fillers):
            state = [[g, n, sp, 0] for g, n, sp in fillers]
            done_m = 0
            for _ in main_gen:
                done_m += 1
                for st in state:
                    g, n, sp, done = st
                    want = min(n, int(n * done_m * sp) // mlen)
                    while st[3] < want and next(g, None) is not None:
                        st[3] += 1
            for g, _, _, _ in state:
                for _ in g:
                    pass

        def emit_all():
            for rc in range(3):
                dispatch_xt(rc)
            for _ in gen_qkv(0):
                pass
            sched = [
                ((0, 0), 3, lambda: [(gen_qkv(1), 28, 2)]),
                ((0, 1), 4, lambda: [(gen_qkv(2), 28, 2), (gen_proj(0, 0), 8, 1)]),
                ((0, 2), 5, lambda: [(gen_qkv(3), 28, 2), (gen_proj(0, 1), 8, 1)]),
                ((0, 3), 6, lambda: [(gen_qkv(4), 28, 2), (gen_proj(0, 2), 8, 1)]),
                ((1, 0), 7, lambda: [(gen_qkv(5), 28, 2)]),
                ((1, 1), None, lambda: [(gen_qkv(6), 28, 2), (gen_proj(0, 3), 8, 1)]),
                ((1, 2), None, lambda: [(gen_qkv(7), 28, 2), (gen_proj(1, 0), 8, 1)]),
                ((1, 3), None, lambda: [(gen_proj(1, 1), 8, 1),
                                        (gen_proj(1, 2), 8, 0.5)]),
            ]
            for (b, j), pre, fills in sched:
                if pre is not None:
                    dispatch_xt(pre)
                nkc = (j + 1) * (QC // KC)
                weave(gen_attn(b, j), 4 * nkc, fills())
            for _ in gen_proj(1, 3, act_evict=True):
                pass

        if loop_n == 1:
            emit_all()
        else:
            with tc.For_i(0, loop_n, 1):
                emit_all()

    nc.compile()
    return nc


_CACHE = {}


def _get_program(loop_n=1):
    key = ("nc", loop_n)
    if key not in _CACHE:
        _CACHE[key] = _build_program(loop_n)
    return _CACHE[key]


def _make_masks():
    k = np.arange(KC, dtype=np.int32)[:, None]
    q = np.arange(KC, dtype=np.int32)[None, :]
    return (q >= k).astype(np.float32)


def make_in_maps(x, Wq, bq, Wk, bk, Wv, bv, Wo):
    bf = mybir.dt.np(BF16)
    x = np.asarray(x, np.float32).reshape(N, D)
    xT = np.ascontiguousarray(x.T)
    masks = np.repeat(_make_masks()[:, None, :], HPC, axis=1).astype(bf)
    Wq, Wk, Wv = (np.asarray(a, np.float32) for a in (Wq, Wk, Wv))
    Wo = np.asarray(Wo, np.float32)
    bq, bk, bv = (np.asarray(a, np.float32) for a in (bq, bk, bv))
    in_maps = []
    for c in range(N_CORES):
        sl = slice(c * CD, (c + 1) * CD)
        in_maps.append({
            "xT": xT,
            "wqkv": np.ascontiguousarray(
                np.stack([Wq[:, sl], Wk[:, sl], Wv[:, sl]], axis=1)),
            "wo": np.ascontiguousarray(Wo[sl, :]).astype(bf),
            "bqkv": np.ascontiguousarray(
                np.stack([bq[sl], bk[sl], bv[sl]], axis=1)),
            "masks": masks,
            "ident": np.eye(KC, dtype=np.float32),
        })
    return in_maps


def _get_runner(loop_n=1):
    """Build (once) a cached jitted SPMD executable over the 8 cores.

    Mirrors concourse.bass2jax.run_bass_via_pjrt's multi-core branch, but
    caches the jitted callable so repeated calls skip re-tracing/compiling,
    and exposes input staging separately so executions can be timed with
    device-resident inputs.
    """
    rkey = ("runner", loop_n)
    if rkey in _CACHE:
        return _CACHE[rkey]
    import jax
    import jax.numpy as jnp
    from jax.sharding import Mesh, PartitionSpec, NamedSharding
    from jax.experimental.shard_map import shard_map
    from concourse import bass2jax
    from concourse import mybir as _mybir

    nc = _get_program(loop_n)
    bass2jax.install_neuronx_cc_hook()

    in_names, out_names, out_avals = [], [], []
    assert nc.dbg_addr is None
    part_name = (nc.partition_id_tensor.name
                 if nc.partition_id_tensor is not None else None)
    for alloc in nc.m.functions[0].allocations:
        if not isinstance(alloc, _mybir.MemoryLocationSet):
            continue
        name = alloc.memorylocations[0].name
        if alloc.kind == "ExternalInput":
            if name != part_name:
                in_names.append(name)
        elif alloc.kind == "ExternalOutput":
            out_names.append(name)
            out_avals.append(jax.core.ShapedArray(
                tuple(alloc.tensor_shape), _mybir.dt.np(alloc.dtype)))
    n_params = len(in_names)
    all_names = in_names + out_names
    if part_name is not None:
        all_names = all_names + [part_name]

    def _body(*args):
        operands = list(args)
        if part_name is not None:
            operands.append(bass2jax.partition_id_tensor())
        return tuple(bass2jax._bass_exec_p.bind(
            *operands,
            out_avals=tuple(out_avals),
            in_names=tuple(all_names),
            out_names=tuple(out_names),
            lowering_input_output_aliases=(),
            sim_require_finite=True,
            sim_require_nnan=True,
            nc=nc,
        ))

    devices = jax.devices()[:N_CORES]
    mesh = Mesh(np.asarray(devices), ("core",))
    nshard = NamedSharding(mesh, PartitionSpec("core"))
    n_outs = len(out_names)
    donate = tuple(range(n_params, n_params + n_outs))
    sharded = jax.jit(
        shard_map(_body, mesh=mesh,
                  in_specs=(PartitionSpec("core"),) * (n_params + n_outs),
                  out_specs=(PartitionSpec("core"),) * n_outs,
                  check_rep=False),
        donate_argnums=donate, keep_unused=True)

    zero_shapes = [(N_CORES * a.shape[0], *a.shape[1:]) for a in out_avals]
    zero_dtypes = [a.dtype for a in out_avals]
    make_zeros = jax.jit(
        lambda: tuple(jnp.zeros(s, d) for s, d in zip(zero_shapes, zero_dtypes)),
        out_shardings=(nshard,) * n_outs)

    def stage(in_maps):
        assert len(in_maps) == N_CORES
        concat = [np.concatenate([np.asarray(m[name]) for m in in_maps], axis=0)
                  for name in in_names]
        return [jax.device_put(a, nshard) for a in concat]

    def execute(staged):
        zeros = make_zeros()
        import jax as _jax
        _jax.block_until_ready(zeros)
        import time as _time
        t0 = _time.perf_counter()
        outs = sharded(*staged, *zeros)
        outs = _jax.block_until_ready(outs)
        dt = _time.perf_counter() - t0
        per_core = [
            {name: np.asarray(outs[i]).reshape(N_CORES, *out_avals[i].shape)[c]
             for i, name in enumerate(out_names)}
            for c in range(N_CORES)]
        return per_core, dt

    _CACHE[rkey] = (stage, execute)
    return _CACHE[rkey]


def run_cores(in_maps):
    """Execute the SPMD program; returns list of per-core {'out': partial}."""
    stage, execute = _get_runner()
    results, _ = execute(stage(in_maps))
    return results


def timed_runs(in_maps, n=8, loop_n=1):
    """Stage inputs once, execute n times, return list of wall durations (s)."""
    stage, execute = _get_runner(loop_n)
    staged = stage(in_maps)
    times = []
    for _ in range(n):
        _, dt = execute(staged)
        times.append(dt)
    return times


def kernel(x, seg, Wq, bq, Wk, bk, Wv, bv, Wo, bo):
    del seg  # unused by the reference computation
    in_maps = make_in_maps(x, Wq, bq, Wk, bk, Wv, bv, Wo)
    results = run_cores(in_maps)
    acc = np.zeros((N, D), np.float32)
    for r in results:
        acc += r["out"].astype(np.float32)
    out = acc + np.asarray(bo, np.float32)
    return out.reshape(B, S, D)
